# revision 41
# baseline (speedup 1.0000x reference)
"""Trainium2 Bass kernel for nn_Attention_light_dwconv_v3 (v2 design).

Data-parallel over batch: 32 batches -> 8 cores x 4 batches. No collectives.

Key differences from the v1 kernel (which did x -> bf16 DRAM -> xbar
transpose -> SBUF and wrote y in f32):
  - x is cast-DMA'd f32->bf16 straight into SBUF token-major, transposed
    on-chip by the PE array (is_transpose matmuls into bf16 PSUM), and
    evicted to fp8(e4m3) channel-major pairs. HBM read traffic drops to
    the 16 MB/batch minimum.
  - y is written bf16 (host casts back to f32): write traffic halves.
  - dwconv runs on the PE as 16 diagonal-matrix matmuls per channel
    chunk (accumulated in PSUM) instead of 16 serial DVE MACs.
  - attention output is computed token-major (attn as the stationary
    operand), so the softmax normalize is a per-partition tensor_scalar
    per (token-tile, head) instead of the denominator-replica chain.
  - q, attn@v and proj matmuls run fp8 DoubleRow (K-pairs block-split
    across the free dim), roughly halving their PE stream time.
Scores stay bf16 (q/k in bf16), LN/pw stay f32: only tensors whose error
averages down over a >=196-wide contraction are fp8.
"""

import os
import sys
from contextlib import ExitStack, nullcontext

import numpy as np

sys.path.insert(0, "/opt/trn_rl_repo")

import ml_dtypes

import concourse.bass as bass
import concourse.mybir as mybir
from concourse import bacc
from concourse.alu_op_type import AluOpType
from concourse.bass_utils import run_bass_kernel_spmd
from concourse.tile import TileContext

BF16 = mybir.dt.bfloat16
F32 = mybir.dt.float32
FP8 = mybir.dt.float8e4
AF = mybir.ActivationFunctionType
DR = mybir.MatmulPerfMode.DoubleRow

B, N, C = 32, 3136, 320
CN, HEADS, DQ, DV = 400, 5, 80, 64
NK = 196  # (56/4)^2
HK = 98   # NK/2 (k-token pair-halves for fp8 DoubleRow)
SR = 4
SCALE = (C // HEADS * 1.25) ** -0.5  # 80^-0.5
NCORES = 8
BL = B // NCORES

# CN=400 chunks for k/v matmul contraction and pw output M-tiles
CNCH = [(0, 128), (128, 128), (256, 128), (384, 16)]
# 512-token groups (also the attention n-tiles): 6x512 + 64
NT = [(i * 512, min(512, N - i * 512)) for i in range((N + 511) // 512)]
# 128-token tiles: 24x128 + 64
TT = [(i * 128, min(128, N - i * 128)) for i in range((N + 127) // 128)]

_built = None
SKIP_ATTN = os.environ.get("K_SKIP_ATTN", "0") == "1"
NO_DR = os.environ.get("K_NO_DR", "1") == "1"
NO_DW = os.environ.get("K_NO_DW", "0") == "1"
SMALL_DMA = os.environ.get("K_SMALL_DMA", "0") == "1"
F32LOAD = os.environ.get("K_F32LOAD", "0") == "1"
DWPE = os.environ.get("K_DWPE", "0") == "1"
A8 = FP8


def build_kernel(reps=1):
    nc = bacc.Bacc("TRN2", target_bir_lowering=False)

    x_in = nc.dram_tensor("x", [BL, N, C], F32, kind="ExternalInput")
    y_out = nc.dram_tensor("y", [BL, N, C], BF16, kind="ExternalOutput")

    w_specs = {
        "ident": ([128, 128], BF16),
        # diag dwconv weights: [ch, tap, ch]
        "dwd0": ([128, 16, 128], BF16), "dwd1": ([128, 16, 128], BF16),
        "dwd2": ([64, 16, 64], BF16),
        # per-partition dwconv taps for the DVE path
        "dwf0": ([128, 16], F32), "dwf1": ([128, 16], F32),
        "dwf2": ([64, 16], F32),
        "qw0": ([128, CN], BF16), "qw1": ([128, CN], BF16),
        "qw2": ([64, CN], BF16),
        "kw0": ([128, CN], BF16), "kw1": ([128, CN], BF16),
        "kw2": ([128, CN], BF16), "kw3": ([16, CN], BF16),
        "vw0": ([128, C], BF16), "vw1": ([128, C], BF16),
        "vw2": ([128, C], BF16), "vw3": ([16, C], BF16),
        "pwt0": ([128, CN], BF16), "pwt1": ([128, CN], BF16), "pwt2": ([65, CN], BF16),
        "prw0": ([128, C], BF16), "prw1": ([128, C], BF16),
        "prw2": ([65, C], BF16),
        "lng": ([128, 4], F32), "lnb": ([128, 4], F32),
    }
    w_dram = {k: nc.dram_tensor(k, sh, dt, kind="ExternalInput")
              for k, (sh, dt) in w_specs.items()}

    with TileContext(nc) as tc, ExitStack() as ctx:
        cpool = ctx.enter_context(tc.tile_pool(name="consts", bufs=1))
        xb_pool = ctx.enter_context(tc.tile_pool(name="xb", bufs=2))
        xq_pool = ctx.enter_context(tc.tile_pool(name="xq", bufs=2))
        sp_pool = ctx.enter_context(tc.tile_pool(name="spatial", bufs=2))
        q_pool = ctx.enter_context(tc.tile_pool(name="qt", bufs=1))
        a_pool = ctx.enter_context(tc.tile_pool(name="attn", bufs=2))
        at_pool = ctx.enter_context(tc.tile_pool(name="atok", bufs=1))
        pr_pool = ctx.enter_context(tc.tile_pool(name="projin", bufs=2))
        y_pool = ctx.enter_context(tc.tile_pool(name="ysb", bufs=2))
        n_pool = ctx.enter_context(tc.tile_pool(name="norm", bufs=3))
        # PSUM: "big" 4KB x2 + prep 2KB x2 + attn/out 2KB x2 = 16KB (the limit)
        ps4 = ctx.enter_context(tc.tile_pool(name="ps4", bufs=2, space="PSUM"))
        ps2 = ctx.enter_context(tc.tile_pool(name="ps2", bufs=2, space="PSUM"))
        ps2o = ctx.enter_context(tc.tile_pool(name="ps2o", bufs=2, space="PSUM"))

        w = {}
        for k, (sh, dt) in w_specs.items():
            w[k] = cpool.tile(sh, dt, tag=k, name=k)
            nc.sync.dma_start(out=w[k], in_=w_dram[k][(slice(None),) * len(sh)])

        ones_sb = cpool.tile([128, 1], BF16, tag="ones")
        nc.vector.memset(ones_sb, 1.0)
        ones_row = cpool.tile([1, 128], F32, tag="ones_row")
        nc.vector.memset(ones_row, 1.0)

        kw = [w["kw0"], w["kw1"], w["kw2"], w["kw3"]]
        vw = [w["vw0"], w["vw1"], w["vw2"], w["vw3"]]
        pwt = [w["pwt0"], w["pwt1"], w["pwt2"]]
        dwd = [w["dwd0"], w["dwd1"], w["dwd2"]]
        ident = w["ident"]

        # x-channel chunks for transpose/dwconv: (c0, cw, which xq, slot)
        XCH = [(0, 128, 0, 0), (128, 128, 0, 1), (256, 64, 1, 0)]

        # eviction-engine round-robin (ACT does the even calls, DVE odd)
        ev_state = [0]

        def ev_copy(out, in_):
            ev_state[0] ^= 1
            if ev_state[0]:
                nc.scalar.copy(out=out, in_=in_)
            else:
                nc.vector.tensor_copy(out=out, in_=in_)

        loop_cm = tc.For_i(0, reps, 1) if reps > 1 else nullcontext()
        with loop_cm:
            for b in range(BL):
                # ---- cast-DMA x f32 -> bf16 SBUF token-major ----
                xb = xb_pool.tile([128, 25, C], BF16, tag="xb")
                if F32LOAD:
                    # plain HWDGE f32 load (full line rate) + DVE cast, in
                    # 4-tile staging chunks: avoids the SWDGE cast-DMA path
                    for g in range(7):
                        t0 = g * 4
                        gsz = min(4, 25 - t0)
                        tokens = min(512, N - t0 * 128)
                        xf = xb_pool.tile([128, 4, C], F32, tag="xf", bufs=2,
                                          name=f"xf{g}")
                        if tokens % 128 == 0:
                            nc.sync.dma_start(
                                out=xf[:, 0:gsz, :],
                                in_=x_in[b, t0 * 128:t0 * 128 + tokens, :]
                                .rearrange("(j p) c -> p j c", p=128))
                            nc.vector.tensor_copy(out=xb[:, t0:t0 + gsz, :],
                                                  in_=xf[:, 0:gsz, :])
                        else:  # tail group: tile 24, 64 tokens
                            nc.sync.dma_start(
                                out=xf[0:64, 0, :],
                                in_=x_in[b, 3072:3136, :])
                            nc.vector.tensor_copy(out=xb[0:64, 24, :],
                                                  in_=xf[0:64, 0, :])
                elif SMALL_DMA:
                    for j0 in range(0, 24, 2):
                        nc.gpsimd.dma_start(
                            out=xb[:, j0:j0 + 2, :],
                            in_=x_in[b, j0 * 128:(j0 + 2) * 128, :]
                            .rearrange("(j p) c -> p j c", p=128))
                else:
                    nc.gpsimd.dma_start(
                        out=xb[:, 0:12, :],
                        in_=x_in[b, 0:1536, :].rearrange("(j p) c -> p j c", p=128))
                    nc.gpsimd.dma_start(
                        out=xb[:, 12:24, :],
                        in_=x_in[b, 1536:3072, :].rearrange("(j p) c -> p j c", p=128))
                if not F32LOAD:
                    nc.gpsimd.dma_start(out=xb[0:64, 24, :],
                                        in_=x_in[b, 3072:3136, :])

                # ---- PE transpose -> xq bf16 (channel-major) ----
                xq = xq_pool.tile([128, 2, N], BF16, tag="xq")
                xq2 = xq_pool.tile([64, N], BF16, tag="xq2")
                for (g0, gw) in NT:
                    ntl = (gw + 127) // 128
                    for (c0, cw, which, slot) in XCH:
                        pt = ps2.tile([128, 1024], BF16, tag="sml", name="pt")
                        for j in range(ntl):
                            t = g0 // 128 + j
                            tw = TT[t][1]
                            nc.tensor.transpose(
                                pt[0:cw, j * 128:j * 128 + tw],
                                xb[0:tw, t, c0:c0 + cw],
                                ident[0:tw, 0:tw])
                        dst = (xq[0:cw, slot, g0:g0 + gw] if which == 0
                               else xq2[0:cw, g0:g0 + gw])
                        ev_copy(dst, pt[0:cw, 0:gw])

                # ---- dwconv on PE: 16 diag matmuls per channel chunk ----
                acc = sp_pool.tile([128, 3, NK], BF16, tag="acc", bufs=1)
                accf = sp_pool.tile([128, 3, NK], F32, tag="accf", bufs=1)
                accf2 = sp_pool.tile([128, 3, NK], F32, tag="accf2", bufs=1)
                if NO_DW:
                    nc.vector.memset(acc[:, :, :], 0.01)
                for ci, (c0, cw, which, slot) in enumerate(XCH):
                    if NO_DW:
                        break
                    src = (xq[0:cw, slot, :] if which == 0 else xq2[0:cw, :])
                    xr = src.rearrange("p (ri a sj b) -> p ri a sj b",
                                       ri=14, a=SR, sj=14, b=SR)
                    if DWPE:
                        pda = ps2.tile([128, 512], F32, tag="sml", name="pda")
                        for tap in range(16):
                            di, dj = tap // SR, tap % SR
                            nc.tensor.matmul(pda[0:cw, 0:NK],
                                             dwd[ci][0:cw, tap, 0:cw],
                                             xr[0:cw, :, di, :, dj],
                                             start=(tap == 0), stop=(tap == 15))
                        nc.vector.tensor_copy(out=acc[0:cw, ci, :],
                                              in_=pda[0:cw, 0:NK])
                        continue
                    # DVE path: two 8-tap chains (keeps PE free for attention)
                    dwf = [w["dwf0"], w["dwf1"], w["dwf2"]][ci]
                    o = accf[0:cw, ci, :].rearrange("p (ri sj) -> p ri sj", sj=14)
                    o2 = accf2[0:cw, ci, :].rearrange("p (ri sj) -> p ri sj", sj=14)
                    for tap in range(16):
                        di, dj = tap // SR, tap % SR
                        sl = xr[0:cw, :, di, :, dj]
                        sc = dwf[0:cw, tap:tap + 1]
                        if tap == 0:
                            nc.vector.tensor_scalar_mul(o, sl, sc)
                        elif tap < 8:
                            nc.vector.scalar_tensor_tensor(
                                out=o, in0=sl, scalar=sc, in1=o,
                                op0=AluOpType.mult, op1=AluOpType.add)
                        elif tap == 8:
                            nc.vector.tensor_scalar_mul(o2, sl, sc)
                        else:
                            nc.vector.scalar_tensor_tensor(
                                out=o2, in0=sl, scalar=sc, in1=o2,
                                op0=AluOpType.mult, op1=AluOpType.add)
                    nc.vector.tensor_tensor(out=acc[0:cw, ci, :],
                                            in0=accf[0:cw, ci, :],
                                            in1=accf2[0:cw, ci, :],
                                            op=AluOpType.add)
                nc.vector.memset(acc[64:65, 2, :], 1.0)  # pw bias ones-row

                # ---- pointwise conv 320->400 (+bias), fp32 matmul ----
                xs_pre = sp_pool.tile([128, 4, NK], BF16, tag="xs_pre", bufs=1)
                xs_sq = sp_pool.tile([128, 4, NK], BF16, tag="xs_sq", bufs=1)
                xsg = sp_pool.tile([128, 4, NK], BF16, tag="xsg", bufs=1)
                for m, (m0, ms) in enumerate(CNCH):
                    pxs = ps2.tile([128, 512], F32, tag="sml", name="pxs")
                    nc.tensor.matmul(pxs[0:ms, 0:NK], pwt[0][:, m0:m0 + ms],
                                     acc[0:128, 0, :], start=True, stop=False)
                    nc.tensor.matmul(pxs[0:ms, 0:NK], pwt[1][:, m0:m0 + ms],
                                     acc[0:128, 1, :], start=False, stop=False)
                    nc.tensor.matmul(pxs[0:ms, 0:NK], pwt[2][0:65, m0:m0 + ms],
                                     acc[0:65, 2, :], start=False, stop=True)
                    nc.vector.tensor_copy(out=xs_pre[0:ms, m, :], in_=pxs[0:ms, 0:NK])
                    nc.scalar.activation(out=xs_sq[0:ms, m, :], in_=pxs[0:ms, 0:NK],
                                         func=AF.Square)

                # ---- layernorm over 400 channels (on partitions) ----
                psum = ps2.tile([128, 512], F32, tag="sml", name="psums")
                for m, (m0, ms) in enumerate(CNCH):
                    nc.tensor.matmul(psum[0:1, 0:NK], ones_sb[0:ms, 0:1],
                                     xs_pre[0:ms, m, :], start=(m == 0),
                                     stop=(m == 3))
                for m, (m0, ms) in enumerate(CNCH):
                    nc.tensor.matmul(psum[32:33, 0:NK], ones_sb[0:ms, 0:1],
                                     xs_sq[0:ms, m, :], start=(m == 0),
                                     stop=(m == 3))
                mr = sp_pool.tile([1, 2, NK], F32, tag="mr", bufs=1)
                vv = sp_pool.tile([1, NK], F32, tag="vv", bufs=1)
                tmp = sp_pool.tile([1, NK], F32, tag="tmp", bufs=1)
                nc.vector.tensor_scalar_mul(mr[0:1, 0, :], psum[0:1, 0:NK], 1.0 / CN)
                nc.vector.tensor_scalar_mul(vv, psum[32:33, 0:NK], 1.0 / CN)
                nc.gpsimd.tensor_tensor(out=tmp, in0=mr[0:1, 0, :], in1=mr[0:1, 0, :],
                                        op=AluOpType.mult)
                nc.gpsimd.tensor_tensor(out=vv, in0=vv, in1=tmp, op=AluOpType.subtract)
                nc.gpsimd.tensor_scalar_add(vv, vv, 1e-5)
                # rstd via mult-only Newton (ACT Sqrt is in another table set)
                yv = mr[0:1, 1, :]
                nc.vector.reciprocal_approx_fast(out=yv, in_=vv)
                nc.gpsimd.tensor_scalar_min(yv, yv, 2.5)
                for _ in range(5):
                    nc.gpsimd.tensor_tensor(out=tmp, in0=yv, in1=yv, op=AluOpType.mult)
                    nc.gpsimd.tensor_tensor(out=tmp, in0=tmp, in1=vv, op=AluOpType.mult)
                    nc.gpsimd.tensor_scalar(out=tmp, in0=tmp, scalar1=-0.5,
                                            scalar2=1.5, op0=AluOpType.mult,
                                            op1=AluOpType.add)
                    nc.gpsimd.tensor_tensor(out=yv, in0=yv, in1=tmp, op=AluOpType.mult)
                pmr = ps2.tile([128, 512], F32, tag="sml", name="pmr")
                nc.tensor.matmul(pmr[:, 0:2 * NK], ones_row[0:1, 0:128],
                                 mr[0:1, :, :].rearrange("p a b -> p (a b)"),
                                 start=True, stop=True)

                # normalize + gelu (tanh approx, stays in Exp act-table set)
                C0, C1 = 0.7978845608028654, 0.044715
                pmr2 = pmr[:, 0:2 * NK].rearrange("p (a b) -> p a b", a=2)
                for m, (m0, ms) in enumerate(CNCH):
                    t = sp_pool.tile([128, NK], F32, tag="normt")
                    s = sp_pool.tile([128, NK], F32, tag="sqt")
                    nc.vector.tensor_tensor(out=t[0:ms, :], in0=xs_pre[0:ms, m, :],
                                            in1=pmr2[0:ms, 0, :],
                                            op=AluOpType.subtract)
                    nc.vector.tensor_tensor(out=t[0:ms, :], in0=t[0:ms, :],
                                            in1=pmr2[0:ms, 1, :],
                                            op=AluOpType.mult)
                    nc.vector.tensor_scalar(out=t[0:ms, :], in0=t[0:ms, :],
                                            scalar1=w["lng"][0:ms, m:m + 1],
                                            scalar2=w["lnb"][0:ms, m:m + 1],
                                            op0=AluOpType.mult, op1=AluOpType.add)
                    nc.scalar.activation(out=s[0:ms, :], in_=t[0:ms, :], func=AF.Square)
                    nc.vector.tensor_scalar(out=s[0:ms, :], in0=s[0:ms, :],
                                            scalar1=C1, scalar2=1.0,
                                            op0=AluOpType.mult, op1=AluOpType.add)
                    nc.vector.tensor_tensor(out=s[0:ms, :], in0=s[0:ms, :],
                                            in1=t[0:ms, :], op=AluOpType.mult)
                    nc.scalar.activation(out=s[0:ms, :], in_=s[0:ms, :], func=AF.Tanh,
                                         scale=C0)
                    nc.vector.tensor_scalar(out=s[0:ms, :], in0=s[0:ms, :],
                                            scalar1=0.5, scalar2=0.5,
                                            op0=AluOpType.mult, op1=AluOpType.add)
                    nc.vector.tensor_tensor(out=xsg[0:ms, m, :], in0=s[0:ms, :],
                                            in1=t[0:ms, :], op=AluOpType.mult)

                # ---- kT [80, 5, 196] bf16 ----
                kT = sp_pool.tile([80, HEADS, NK], BF16, tag="kT")
                for h in range(HEADS):
                    pk = ps2.tile([128, 512], F32, tag="sml", name="pk")
                    for m, (m0, ms) in enumerate(CNCH):
                        nc.tensor.matmul(pk[0:DQ, 0:NK],
                                         kw[m][0:ms, DQ * h:DQ * (h + 1)],
                                         xsg[0:ms, m, :], start=(m == 0),
                                         stop=(m == 3))
                    ev_copy(kT[:, h, :], pk[0:DQ, 0:NK])

                if SKIP_ATTN:
                    # bisection stub: y <- x (exercises prep + y-DMA only)
                    for t, (t0, tw) in enumerate(TT):
                        if t % 5 == 0:
                            ysb = y_pool.tile([128, 5, C], BF16, tag="ysb",
                                              name=f"ysbS{t}")
                        ev_copy(ysb[0:tw, t % 5, :], xb[0:tw, t, :])
                        if t % 5 == 4 or t == len(TT) - 1:
                            gbase = (t // 5) * 640
                            gtok = min(640, N - gbase)
                            full = gtok // 128
                            if full:
                                nc.sync.dma_start(
                                    out=y_out[b, gbase:gbase + full * 128, :]
                                    .rearrange("(j p) c -> p j c", p=128),
                                    in_=ysb[:, 0:full, :])
                            if gtok % 128:
                                nc.sync.dma_start(
                                    out=y_out[b, gbase + full * 128:gbase + gtok, :],
                                    in_=ysb[0:gtok % 128, full, :])
                    continue

                # ---- v_aug [98, 5, 2, 65]: per head [v(64) | ones(1)] ----
                # head-major so the DR rhs slice [0:HK, h, :, :] is contiguous
                v_aug = sp_pool.tile([HK, HEADS, 2, 65], A8, tag="vaug")
                for ti in range(2):
                    t0 = ti * HK
                    pv = ps2.tile([128, 512], F32, tag="sml", name="pv")
                    for m, (m0, ms) in enumerate(CNCH):
                        nc.tensor.matmul(pv[0:HK, 0:C], xsg[0:ms, m, t0:t0 + HK],
                                         vw[m][0:ms, :], start=(m == 0),
                                         stop=(m == 3))
                    for h in range(HEADS):
                        ev_copy(v_aug[0:HK, h, ti, 0:64],
                                pv[0:HK, DV * h:DV * (h + 1)])
                nc.vector.memset(v_aug[0:HK, :, :, 64:65], 1.0)

                # ---- qT [80, 5, 3136] bf16 ----
                qT = q_pool.tile([DQ, HEADS, N], FP8, tag="qT")
                for h in range(HEADS):
                    for p in range(0, len(NT), 2):
                        pq = ps4.tile([128, 2, 512], F32, tag="big", name="pq")
                        nsub = min(2, len(NT) - p)
                        for si in range(nsub):
                            nt0, ntw = NT[p + si]
                            nc.tensor.matmul(
                                pq[0:DQ, si, 0:ntw],
                                w["qw0"][:, DQ * h:DQ * (h + 1)],
                                xq[:, 0, nt0:nt0 + ntw],
                                start=True, stop=False)
                            nc.tensor.matmul(
                                pq[0:DQ, si, 0:ntw],
                                w["qw1"][:, DQ * h:DQ * (h + 1)],
                                xq[:, 1, nt0:nt0 + ntw],
                                start=False, stop=False)
                            nc.tensor.matmul(
                                pq[0:DQ, si, 0:ntw],
                                w["qw2"][:, DQ * h:DQ * (h + 1)],
                                xq2[:, nt0:nt0 + ntw],
                                start=False, stop=True)
                        nt0 = NT[p][0]
                        tot = sum(NT[p + si][1] for si in range(nsub))
                        if tot == 1024:
                            ev_copy(qT[:, h, nt0:nt0 + 1024],
                                    pq[0:DQ, :, :].rearrange("p a b -> p (a b)"))
                        else:
                            for si in range(nsub):
                                s0, sw = NT[p + si]
                                ev_copy(qT[:, h, s0:s0 + sw], pq[0:DQ, si, 0:sw])

                # ---- attention ----
                att_tok = at_pool.tile([128, 25, HEADS, DV], BF16, tag="atok")
                for (nt0, ntw) in NT:
                    a_all = a_pool.tile([HK, HEADS, 2, 512], A8, tag="a_all")
                    for h in range(HEADS):
                        pst = ps4.tile([128, 2, 512], F32, tag="big", name="pst")
                        nc.tensor.matmul(pst[0:HK, 0, 0:ntw], kT[:, h, 0:HK],
                                         qT[:, h, nt0:nt0 + ntw],
                                         start=True, stop=True)
                        nc.tensor.matmul(pst[0:HK, 1, 0:ntw], kT[:, h, HK:NK],
                                         qT[:, h, nt0:nt0 + ntw],
                                         start=True, stop=True)
                        nc.scalar.activation(out=a_all[0:HK, h, :, 0:ntw],
                                             in_=pst[0:HK, :, 0:ntw],
                                             func=AF.Exp, scale=SCALE)
                    for j in range((ntw + 127) // 128):
                        t = nt0 // 128 + j
                        tw = TT[t][1]
                        j0 = j * 128
                        pav = ps2o.tile([128, 512], F32, tag="smlo", name="pav")
                        pav5 = pav[:, 0:HEADS * 65].rearrange("p (h e) -> p h e",
                                                              e=65)
                        for h in range(HEADS):
                            if NO_DR:
                                nc.tensor.matmul(pav5[0:tw, h, :],
                                                 a_all[0:HK, h, 0, j0:j0 + tw],
                                                 v_aug[0:HK, h, 0, :],
                                                 start=True, stop=False)
                                nc.tensor.matmul(pav5[0:tw, h, :],
                                                 a_all[0:HK, h, 1, j0:j0 + tw],
                                                 v_aug[0:HK, h, 1, :],
                                                 start=False, stop=True)
                            else:
                                nc.tensor.matmul(pav5[0:tw, h, :],
                                                 a_all[0:HK, h, :, j0:j0 + tw],
                                                 v_aug[0:HK, h, :, :],
                                                 start=True, stop=True,
                                                 perf_mode=DR)
                        den = n_pool.tile([128, 8], F32, tag="den")
                        rec = n_pool.tile([128, 8], F32, tag="rec")
                        nc.scalar.copy(out=den[0:tw, 0:HEADS],
                                       in_=pav5[0:tw, :, 64])
                        nc.vector.reciprocal_approx_fast(out=rec[0:tw, 0:HEADS],
                                                         in_=den[0:tw, 0:HEADS])
                        for h in range(HEADS):
                            nc.vector.tensor_scalar_mul(att_tok[0:tw, t, h, :],
                                                        pav5[0:tw, h, 0:64],
                                                        rec[0:tw, h:h + 1])

                # ---- transpose att_tok -> attT bf16 (channel-major) ----
                attT01 = pr_pool.tile([128, 2, N], BF16, tag="attT01")
                attT2 = pr_pool.tile([65, N], BF16, tag="attT2")
                atv = att_tok.rearrange("p t h e -> p t (h e)")
                for (g0, gw) in NT:
                    ntl = (gw + 127) // 128
                    for (c0, cw, which, slot) in XCH:
                        pt = ps2o.tile([128, 1024], BF16, tag="smlo", name="ptA")
                        for j in range(ntl):
                            t = g0 // 128 + j
                            tw = TT[t][1]
                            nc.tensor.transpose(
                                pt[0:cw, j * 128:j * 128 + tw],
                                atv[0:tw, t, c0:c0 + cw],
                                ident[0:tw, 0:tw])
                        dst = (attT01[0:cw, slot, g0:g0 + gw] if which == 0
                               else attT2[0:cw, g0:g0 + gw])
                        ev_copy(dst, pt[0:cw, 0:gw])
                nc.vector.memset(attT2[64:65, :], 1.0)  # proj bias ones-row

                # ---- proj (fp8 DoubleRow + bias row) -> y bf16 ----
                ysb = None
                for t, (t0, tw) in enumerate(TT):
                    if t % 5 == 0:
                        ysb = y_pool.tile([128, 5, C], BF16, tag="ysb",
                                          name=f"ysb{t}")
                    py = ps2o.tile([128, 512], F32, tag="smlo", name="py")
                    nc.tensor.matmul(py[0:tw, 0:C], attT01[:, 0, t0:t0 + tw],
                                     w["prw0"][:, :], start=True, stop=False)
                    nc.tensor.matmul(py[0:tw, 0:C], attT01[:, 1, t0:t0 + tw],
                                     w["prw1"][:, :], start=False, stop=False)
                    nc.tensor.matmul(py[0:tw, 0:C], attT2[:, t0:t0 + tw],
                                     w["prw2"][:, :], start=False, stop=True)
                    ev_copy(ysb[0:tw, t % 5, :], py[0:tw, 0:C])
                    if t % 5 == 4 or t == len(TT) - 1:
                        gbase = (t // 5) * 640
                        gtok = min(640, N - gbase)
                        full = gtok // 128
                        if full:
                            nc.sync.dma_start(
                                out=y_out[b, gbase:gbase + full * 128, :]
                                .rearrange("(j p) c -> p j c", p=128),
                                in_=ysb[:, 0:full, :])
                        if gtok % 128:
                            nc.sync.dma_start(
                                out=y_out[b, gbase + full * 128:gbase + gtok, :],
                                in_=ysb[0:gtok % 128, full, :])

    nc.finalize()
    return nc


def _prep_weights(dw_w, dw_b, pw_w, pw_b, ln_g, ln_b, q_w, k_w, v_w,
                  proj_w, proj_b):
    bf = ml_dtypes.bfloat16
    f8 = ml_dtypes.float8_e4m3
    f = np.float32

    def to8(a):
        return np.clip(np.asarray(a, f), -240.0, 240.0).astype(f8)

    dw_w, dw_b = np.asarray(dw_w, f), np.asarray(dw_b, f)
    pw_w, pw_b = np.asarray(pw_w, f), np.asarray(pw_b, f)
    ln_g, ln_b = np.asarray(ln_g, f), np.asarray(ln_b, f)
    q_w, k_w, v_w = np.asarray(q_w, f), np.asarray(k_w, f), np.asarray(v_w, f)
    proj_w, proj_b = np.asarray(proj_w, f), np.asarray(proj_b, f)

    out = {}
    out["ident"] = np.eye(128, dtype=bf)
    dwf = dw_w.reshape(C, 16)  # [320, 16]
    for ci, (c0, cw) in enumerate([(0, 128), (128, 128), (256, 64)]):
        d = np.zeros((cw, 16, cw), np.float32)
        d[np.arange(cw), :, np.arange(cw)] = dwf[c0:c0 + cw]
        out[f"dwd{ci}"] = d.astype(bf)
        out[f"dwf{ci}"] = dwf[c0:c0 + cw].astype(f)
    out["qw0"] = q_w[0:128].astype(bf)
    out["qw1"] = q_w[128:256].astype(bf)
    out["qw2"] = q_w[256:320].astype(bf)
    for i, (r0, rs) in enumerate(CNCH):
        out[f"kw{i}"] = k_w[r0:r0 + rs].astype(bf)
        out[f"vw{i}"] = v_w[r0:r0 + rs].astype(bf)
    pwt = np.ascontiguousarray(pw_w.T)  # [320, 400]
    pw_b_eff = pw_b + pw_w @ dw_b
    out["pwt0"] = pwt[0:128].astype(bf)
    out["pwt1"] = pwt[128:256].astype(bf)
    out["pwt2"] = np.concatenate([pwt[256:320], pw_b_eff[None, :]], 0).astype(bf)
    out["prw0"] = proj_w[0:128].astype(bf)
    out["prw1"] = proj_w[128:256].astype(bf)
    out["prw2"] = np.concatenate([proj_w[256:320], proj_b[None, :]], 0).astype(bf)
    lng = np.zeros((128, 4), f)
    lnb = np.zeros((128, 4), f)
    for m, (m0, ms) in enumerate(CNCH):
        lng[0:ms, m] = ln_g[m0:m0 + ms]
        lnb[0:ms, m] = ln_b[m0:m0 + ms]
    out["lng"], out["lnb"] = lng, lnb
    return out


LAST_RESULT = None


def kernel(x, H, W, dw_w, dw_b, pw_w, pw_b, ln_g, ln_b, q_w, k_w, v_w,
           proj_w, proj_b):
    global _built, LAST_RESULT
    assert int(H) == 56 and int(W) == 56
    x = np.asarray(x, np.float32)
    assert x.shape == (B, N, C), x.shape

    if _built is None:
        _built = build_kernel()
    nc = _built

    wmaps = _prep_weights(dw_w, dw_b, pw_w, pw_b, ln_g, ln_b, q_w, k_w, v_w,
                          proj_w, proj_b)
    in_maps = []
    for c in range(NCORES):
        m = {"x": np.ascontiguousarray(x[c * BL:(c + 1) * BL])}
        m.update(wmaps)
        in_maps.append(m)

    trace = os.environ.get("KERNEL_TRACE", "0") == "1"
    res = run_bass_kernel_spmd(nc, in_maps, core_ids=list(range(NCORES)),
                               trace=trace)
    LAST_RESULT = res
    y = np.concatenate([np.asarray(r["y"]) for r in res.results], axis=0)
    return y.astype(np.float32)


if __name__ == "__main__":
    print("smoke test: building kernel IR only")
    nc = build_kernel()
    print("built OK")
